# revision 14
# baseline (speedup 1.0000x reference)
"""BiRealLinear Trainium2 kernel (host-tiled operands + fp8 DoubleRow).

Computes out = binact(x) @ quant_weight(w).T for
  x [4, 2048, 4096] f32, w [4096, 4096] f32  ->  out [4, 2048, 4096] f32

Forward semantics (STE parts drop out in forward):
  binact(x)       = sign(x)                      in {-1, 0, +1}
  quant_weight(w) = mean(|w|, axis=1) * sign(w)  per-output-row scale

So out[t, o] = scale[o] * sum_i sign(x[t,i]) * sign(w[o,i]).

8 cores in a 4 (token) x 2 (out-feature) grid; each core does a
[2048 x 4096 x 2048] sign-matmul. Host supplies both operands already
transposed (contraction dim i outermost) as bf16 k-tiles [32, 128, T] --
bf16 transport is exact for sign() and loses <1e-4 on mean|w| -- so the
device does no transposes at all. On-device per core:
  - plain contiguous DMA loads of the k-tiles,
  - ACT sign -> fp8e4 (+/-1 exact) pair-tiles [128, 2, T],
  - |w| = w * sign(w) (exact in bf16) summed into f32 partials for the
    scale row (finished with two tiny PE reductions/broadcasts),
  - PE matmul in fp8 DoubleRow mode: each MM contracts 256 (two k-tiles),
    PSUM f32 accumulation is exact (integer sums <= 4096),
  - contraction split in 2 chunks of 2048 with an exact fp16 SBUF
    accumulator so matmuls start before all operands are loaded,
  - drain = (chunk0_acc + PSUM) * scale_bc -> f32 out tiles -> DMA.
"""

import sys

import numpy as np

try:
    import concourse.bacc as bacc  # noqa: F401
except ImportError:
    sys.path.insert(0, "/opt/trn_rl_repo")

import ml_dtypes

import concourse.bacc as bacc
import concourse.mybir as mybir
import concourse.tile as tile
from concourse.bass_utils import run_bass_kernel_spmd

dt = mybir.dt

# ---- problem geometry (hardcoded; full input is [8192, 4096] x [4096, 4096])
B, S, I_FULL, O_FULL = 4, 2048, 4096, 4096
T_FULL = B * S                      # 8192 tokens
T_GRID, O_GRID = 4, 2               # core grid: 4 token shards x 2 out shards
T_SH = T_FULL // T_GRID             # 2048 tokens per core
O_SH = O_FULL // O_GRID             # 2048 out features per core

P = 128                             # partitions
NK = I_FULL // P                    # 32 k-tiles
NKP = NK // 2                       # 16 k-pairs (DoubleRow contracts 2 tiles)
NCH = 2                             # contraction chunks (k-pairs 0-7 / 8-15)
KPC = NKP // NCH                    # 8 k-pairs per chunk
NTH = 2                             # token halves
TH = T_SH // NTH                    # 1024 tokens per half
NTB = T_SH // P                     # 16 t-blocks
TBH = NTB // NTH                    # 8 t-blocks per half
OP = 512                            # output panel (one PSUM bank)
NPAN = O_SH // OP                   # 4 panels


def build_nc():
    nc = bacc.Bacc("TRN2", target_bir_lowering=False, debug=False, num_devices=8)
    x = nc.dram_tensor("x", [NK, P, T_SH], dt.bfloat16, kind="ExternalInput")
    w = nc.dram_tensor("w", [NK, P, O_SH], dt.bfloat16, kind="ExternalInput")
    out = nc.dram_tensor("out", [T_SH, O_SH], dt.float32, kind="ExternalOutput")

    with tile.TileContext(nc) as tc:
        with (
            tc.tile_pool(name="single", bufs=1) as sb,
            tc.tile_pool(name="wstg", bufs=4) as wstg,
            tc.tile_pool(name="xstg", bufs=8) as xstg,
            tc.tile_pool(name="wabs", bufs=2) as wabp,
            tc.tile_pool(name="wp", bufs=NKP) as wpp,
            tc.tile_pool(name="xp", bufs=20) as xpp,
            tc.tile_pool(name="acc", bufs=TBH) as accp,
            tc.tile_pool(name="ostg", bufs=6) as ostgp,
            tc.tile_pool(name="ps", bufs=6, space="PSUM") as psp,
            tc.tile_pool(name="pss", bufs=2, space="PSUM") as pssp,
        ):
            ones_inv = sb.tile([P, 1], dt.bfloat16)     # 1/I column (k-reduce)
            nc.vector.memset(ones_inv[:], 1.0 / I_FULL)
            ones_row = sb.tile([1, P], dt.float32)      # 1.0 row (broadcast)
            nc.vector.memset(ones_row[:], 1.0)
            srow = sb.tile([1, O_SH], dt.float32)       # scale row
            scale_bc = sb.tile([P, O_SH], dt.float32)   # scale bcast to 128p
            # |w| column sums accumulate in ONE psum bank: the reduce matmul
            # for o-panel p writes M=1 rows at partition 32p (tile_position).
            pscale = pssp.tile([P, OP], dt.float32, tag="pss", name="pscale")

            WP = [None] * NKP                 # fp8 w pair-tiles, all resident
            XP = {}                           # (th, kp) -> fp8 x pair-tile

            def load_w_pair(kp):
                wp_t = wpp.tile([P, 2, O_SH], dt.float8e4, tag="wp",
                                name=f"wp_{kp}")
                for j in range(2):
                    k = 2 * kp + j
                    wtb = wstg.tile([P, O_SH], dt.bfloat16, tag="wstg",
                                    name=f"wtb_{k}")
                    nc.sync.dma_start(wtb[:], w[k])
                    nc.scalar.sign(wp_t[:, j, :], wtb[:])
                    wab = wabp.tile([P, O_SH], dt.bfloat16, tag="wabs",
                                    name=f"wab_{k}")
                    nc.vector.tensor_mul(wab[:], wtb[:], wp_t[:, j, :])
                    # scale partials: pscale[32p, o] += (1/I) * sum_i |w|
                    for p in range(NPAN):
                        nc.tensor.matmul(
                            pscale[32 * p:32 * p + 1, :],
                            lhsT=ones_inv[:],
                            rhs=wab[:, p * OP:(p + 1) * OP],
                            start=(k == 0), stop=(k == NK - 1),
                            tile_position=(0, 32 * p),
                            skip_group_check=True,
                        )
                WP[kp] = wp_t

            def load_x_pair(th, kp):
                xp_t = xpp.tile([P, 2, TH], dt.float8e4, tag="xp",
                                name=f"xp_{th}_{kp}")
                for j in range(2):
                    k = 2 * kp + j
                    xtb = xstg.tile([P, TH], dt.bfloat16, tag="xstg",
                                    name=f"xtb_{th}_{k}")
                    nc.sync.dma_start(xtb[:], x[k][:, th * TH:(th + 1) * TH])
                    nc.scalar.sign(xp_t[:, j, :], xtb[:])
                XP[(th, kp)] = xp_t

            def compute_scale():
                # pscale rows 0/32/64/96 hold the per-panel scale rows;
                # broadcast each to 128 partitions via a K=1 ones matmul.
                for p in range(NPAN):
                    nc.scalar.copy(srow[:, p * OP:(p + 1) * OP],
                                   pscale[32 * p:32 * p + 1, :])
                for p in range(NPAN):
                    psb = pssp.tile([P, OP], dt.float32, tag="pss",
                                    name=f"pssb_{p}")
                    nc.tensor.matmul(psb[:], lhsT=ones_row[:],
                                     rhs=srow[0:1, p * OP:(p + 1) * OP],
                                     start=True, stop=True)
                    nc.vector.tensor_copy(scale_bc[:, p * OP:(p + 1) * OP],
                                          psb[:])

            def mm_chunk(th, c, accs):
                first, last = (c == 0), (c == NCH - 1)
                for tbl in range(TBH):
                    tb = th * TBH + tbl
                    pst = [psp.tile([P, OP], dt.float32, tag="ps",
                                    name=f"ps_{tb}_{c}_{p}")
                           for p in range(NPAN)]
                    for kpl in range(KPC):
                        kp = c * KPC + kpl
                        lhsT = XP[(th, kp)][:, :, tbl * P:(tbl + 1) * P]
                        for p in range(NPAN):
                            nc.tensor.matmul(
                                pst[p][:], lhsT=lhsT,
                                rhs=WP[kp][:, :, p * OP:(p + 1) * OP],
                                start=(kpl == 0), stop=(kpl == KPC - 1),
                                perf_mode=mybir.MatmulPerfMode.DoubleRow,
                            )
                    if first:
                        acc = accp.tile([P, O_SH], dt.float16, tag="acc",
                                        name=f"acc_{tb}")
                        accs[tbl] = acc
                        for p in range(NPAN):
                            nc.vector.tensor_copy(acc[:, p * OP:(p + 1) * OP],
                                                  pst[p][:])
                    if last:
                        acc = accs[tbl]
                        for p in range(NPAN):
                            og = ostgp.tile([P, OP], dt.float32, tag="ostg",
                                            name=f"og_{tb}_{p}")
                            nc.vector.tensor_add(og[:], pst[p][:],
                                                 acc[:, p * OP:(p + 1) * OP])
                            nc.vector.tensor_mul(og[:], og[:],
                                                 scale_bc[:, p * OP:(p + 1) * OP])
                            nc.scalar.dma_start(
                                out[tb * P:(tb + 1) * P, p * OP:(p + 1) * OP],
                                og[:])

            # ---- program order: loads/signs feed the PE just ahead of use.
            # PE order is [minis c0][A-c0][minis c1][bcast][A-c1][B-c0][B-c1]
            # so the scale minis backfill the ACT-gated load window and the
            # chunk-1 minis/loads issue behind the A-c0 mains.
            for kp in range(KPC):             # chunk 0 loads + signs + minis
                load_w_pair(kp)
                load_x_pair(0, kp)
            accs_a = [None] * TBH
            mm_chunk(0, 0, accs_a)
            for kp in range(KPC, NKP):        # chunk 1 loads + signs + minis
                load_w_pair(kp)
                load_x_pair(0, kp)
            compute_scale()
            for kp in range(KPC):             # prefetch x th1 chunk0
                load_x_pair(1, kp)
            mm_chunk(0, 1, accs_a)
            for kp in range(KPC, NKP):        # prefetch x th1 chunk1
                load_x_pair(1, kp)
            accs_b = [None] * TBH
            mm_chunk(1, 0, accs_b)
            mm_chunk(1, 1, accs_b)

    nc.compile()
    return nc


_NC_CACHE = None


def _get_nc():
    global _NC_CACHE
    if _NC_CACHE is None:
        _NC_CACHE = build_nc()
    return _NC_CACHE


def make_in_maps(x, weight):
    """Host-side shard + layout prep: per-core transposed bf16 k-tiles."""
    bf16 = ml_dtypes.bfloat16
    x = np.asarray(x, dtype=np.float32).reshape(T_FULL, I_FULL)
    weight = np.asarray(weight, dtype=np.float32)
    xts = []
    for ti in range(T_GRID):
        sh = x[ti * T_SH:(ti + 1) * T_SH]                 # [2048, 4096]
        xts.append(sh.T.astype(bf16, order="C").reshape(NK, P, T_SH))
    wts = []
    for oj in range(O_GRID):
        sh = weight[oj * O_SH:(oj + 1) * O_SH]
        wts.append(sh.T.astype(bf16, order="C").reshape(NK, P, O_SH))
    in_maps = []
    for core in range(8):
        ti, oj = core // O_GRID, core % O_GRID
        in_maps.append({"x": xts[ti], "w": wts[oj]})
    return in_maps


def kernel(x, weight):
    in_maps = make_in_maps(x, weight)
    nc = _get_nc()
    res = run_bass_kernel_spmd(nc, in_maps, list(range(8)))
    out = np.empty((T_FULL, O_FULL), dtype=np.float32)
    for core in range(8):
        ti, oj = core // O_GRID, core % O_GRID
        out[ti * T_SH:(ti + 1) * T_SH, oj * O_SH:(oj + 1) * O_SH] = (
            res.results[core]["out"]
        )
    return out.reshape(B, S, O_FULL)


# revision 15
# speedup vs baseline: 1.0014x; 1.0014x over previous
"""BiRealLinear Trainium2 kernel (host-tiled operands + fp8 DoubleRow).

Computes out = binact(x) @ quant_weight(w).T for
  x [4, 2048, 4096] f32, w [4096, 4096] f32  ->  out [4, 2048, 4096] f32

Forward semantics (STE parts drop out in forward):
  binact(x)       = sign(x)                      in {-1, 0, +1}
  quant_weight(w) = mean(|w|, axis=1) * sign(w)  per-output-row scale

So out[t, o] = scale[o] * sum_i sign(x[t,i]) * sign(w[o,i]).

8 cores in a 4 (token) x 2 (out-feature) grid; each core does a
[2048 x 4096 x 2048] sign-matmul. Host supplies both operands already
transposed (contraction dim i outermost) as bf16 k-tiles [32, 128, T] --
bf16 transport is exact for sign() and loses <1e-4 on mean|w| -- so the
device does no transposes at all. On-device per core:
  - plain contiguous DMA loads of the k-tiles,
  - ACT sign -> fp8e4 (+/-1 exact) pair-tiles [128, 2, T],
  - |w| = w * sign(w) (exact in bf16) summed into f32 partials for the
    scale row (finished with two tiny PE reductions/broadcasts),
  - PE matmul in fp8 DoubleRow mode: each MM contracts 256 (two k-tiles),
    PSUM f32 accumulation is exact (integer sums <= 4096),
  - contraction split in 2 chunks of 2048 with an exact fp16 SBUF
    accumulator so matmuls start before all operands are loaded,
  - drain = (chunk0_acc + PSUM) * scale_bc -> f32 out tiles -> DMA.
"""

import sys

import numpy as np

try:
    import concourse.bacc as bacc  # noqa: F401
except ImportError:
    sys.path.insert(0, "/opt/trn_rl_repo")

import ml_dtypes

import concourse.bacc as bacc
import concourse.mybir as mybir
import concourse.tile as tile
from concourse.bass_utils import run_bass_kernel_spmd

dt = mybir.dt

# ---- problem geometry (hardcoded; full input is [8192, 4096] x [4096, 4096])
B, S, I_FULL, O_FULL = 4, 2048, 4096, 4096
T_FULL = B * S                      # 8192 tokens
T_GRID, O_GRID = 4, 2               # core grid: 4 token shards x 2 out shards
T_SH = T_FULL // T_GRID             # 2048 tokens per core
O_SH = O_FULL // O_GRID             # 2048 out features per core

P = 128                             # partitions
NK = I_FULL // P                    # 32 k-tiles
NKP = NK // 2                       # 16 k-pairs (DoubleRow contracts 2 tiles)
NCH = 2                             # contraction chunks (k-pairs 0-7 / 8-15)
KPC = NKP // NCH                    # 8 k-pairs per chunk
NTH = 2                             # token halves
TH = T_SH // NTH                    # 1024 tokens per half
NTB = T_SH // P                     # 16 t-blocks
TBH = NTB // NTH                    # 8 t-blocks per half
OP = 512                            # output panel (one PSUM bank)
NPAN = O_SH // OP                   # 4 panels


def build_nc():
    nc = bacc.Bacc("TRN2", target_bir_lowering=False, debug=False, num_devices=8)
    x = nc.dram_tensor("x", [NK, P, T_SH], dt.bfloat16, kind="ExternalInput")
    w = nc.dram_tensor("w", [NK, P, O_SH], dt.bfloat16, kind="ExternalInput")
    out = nc.dram_tensor("out", [T_SH, O_SH], dt.float32, kind="ExternalOutput")

    with tile.TileContext(nc) as tc:
        with (
            tc.tile_pool(name="single", bufs=1) as sb,
            tc.tile_pool(name="wstg", bufs=4) as wstg,
            tc.tile_pool(name="xstg", bufs=4) as xstg,
            tc.tile_pool(name="wabs", bufs=2) as wabp,
            tc.tile_pool(name="wp", bufs=NKP) as wpp,
            tc.tile_pool(name="xp", bufs=20) as xpp,
            tc.tile_pool(name="acc", bufs=TBH) as accp,
            tc.tile_pool(name="ostg", bufs=6) as ostgp,
            tc.tile_pool(name="ps", bufs=6, space="PSUM") as psp,
            tc.tile_pool(name="pss", bufs=2, space="PSUM") as pssp,
        ):
            ones_inv = sb.tile([P, 1], dt.bfloat16)     # 1/I column (k-reduce)
            nc.vector.memset(ones_inv[:], 1.0 / I_FULL)
            ones_row = sb.tile([1, P], dt.float32)      # 1.0 row (broadcast)
            nc.vector.memset(ones_row[:], 1.0)
            srow = sb.tile([1, O_SH], dt.float32)       # scale row
            scale_bc = sb.tile([P, O_SH], dt.float32)   # scale bcast to 128p
            # |w| column sums accumulate in ONE psum bank: the reduce matmul
            # for o-panel p writes M=1 rows at partition 32p (tile_position).
            pscale = pssp.tile([P, OP], dt.float32, tag="pss", name="pscale")

            WP = [None] * NKP                 # fp8 w pair-tiles, all resident
            XP = {}                           # (th, kp) -> fp8 x pair-tile

            def load_w_pair(kp):
                wp_t = wpp.tile([P, 2, O_SH], dt.float8e4, tag="wp",
                                name=f"wp_{kp}")
                for j in range(2):
                    k = 2 * kp + j
                    wtb = wstg.tile([P, O_SH], dt.bfloat16, tag="wstg",
                                    name=f"wtb_{k}")
                    nc.sync.dma_start(wtb[:], w[k])
                    nc.scalar.sign(wp_t[:, j, :], wtb[:])
                    wab = wabp.tile([P, O_SH], dt.bfloat16, tag="wabs",
                                    name=f"wab_{k}")
                    nc.vector.tensor_mul(wab[:], wtb[:], wp_t[:, j, :])
                    # scale partials: pscale[32p, o] += (1/I) * sum_i |w|
                    for p in range(NPAN):
                        nc.tensor.matmul(
                            pscale[32 * p:32 * p + 1, :],
                            lhsT=ones_inv[:],
                            rhs=wab[:, p * OP:(p + 1) * OP],
                            start=(k == 0), stop=(k == NK - 1),
                            tile_position=(0, 32 * p),
                            skip_group_check=True,
                        )
                WP[kp] = wp_t

            def load_x_pair(th, kp):
                xp_t = xpp.tile([P, 2, TH], dt.float8e4, tag="xp",
                                name=f"xp_{th}_{kp}")
                for j in range(2):
                    k = 2 * kp + j
                    xtb = xstg.tile([P, TH], dt.bfloat16, tag="xstg",
                                    name=f"xtb_{th}_{k}")
                    nc.sync.dma_start(xtb[:], x[k][:, th * TH:(th + 1) * TH])
                    nc.scalar.sign(xp_t[:, j, :], xtb[:])
                XP[(th, kp)] = xp_t

            def compute_scale():
                # pscale rows 0/32/64/96 hold the per-panel scale rows;
                # broadcast each to 128 partitions via a K=1 ones matmul.
                for p in range(NPAN):
                    nc.scalar.copy(srow[:, p * OP:(p + 1) * OP],
                                   pscale[32 * p:32 * p + 1, :])
                for p in range(NPAN):
                    psb = pssp.tile([P, OP], dt.float32, tag="pss",
                                    name=f"pssb_{p}")
                    nc.tensor.matmul(psb[:], lhsT=ones_row[:],
                                     rhs=srow[0:1, p * OP:(p + 1) * OP],
                                     start=True, stop=True)
                    nc.vector.tensor_copy(scale_bc[:, p * OP:(p + 1) * OP],
                                          psb[:])

            def mm_chunk(th, c, accs):
                first, last = (c == 0), (c == NCH - 1)
                for tbl in range(TBH):
                    tb = th * TBH + tbl
                    pst = [psp.tile([P, OP], dt.float32, tag="ps",
                                    name=f"ps_{tb}_{c}_{p}")
                           for p in range(NPAN)]
                    for kpl in range(KPC):
                        kp = c * KPC + kpl
                        lhsT = XP[(th, kp)][:, :, tbl * P:(tbl + 1) * P]
                        for p in range(NPAN):
                            nc.tensor.matmul(
                                pst[p][:], lhsT=lhsT,
                                rhs=WP[kp][:, :, p * OP:(p + 1) * OP],
                                start=(kpl == 0), stop=(kpl == KPC - 1),
                                perf_mode=mybir.MatmulPerfMode.DoubleRow,
                            )
                    if first:
                        acc = accp.tile([P, O_SH], dt.float16, tag="acc",
                                        name=f"acc_{tb}")
                        accs[tbl] = acc
                        for p in range(NPAN):
                            nc.vector.tensor_copy(acc[:, p * OP:(p + 1) * OP],
                                                  pst[p][:])
                    if last:
                        acc = accs[tbl]
                        for p in range(NPAN):
                            og = ostgp.tile([P, OP], dt.float32, tag="ostg",
                                            name=f"og_{tb}_{p}")
                            nc.vector.tensor_add(og[:], pst[p][:],
                                                 acc[:, p * OP:(p + 1) * OP])
                            nc.vector.tensor_mul(og[:], og[:],
                                                 scale_bc[:, p * OP:(p + 1) * OP])
                            nc.scalar.dma_start(
                                out[tb * P:(tb + 1) * P, p * OP:(p + 1) * OP],
                                og[:])

            # ---- program order: loads/signs feed the PE just ahead of use.
            # PE order is [minis c0][A-c0][minis c1][bcast][A-c1][B-c0][B-c1]
            # so the scale minis backfill the ACT-gated load window and the
            # chunk-1 minis/loads issue behind the A-c0 mains.
            for kp in range(KPC):             # chunk 0 loads + signs + minis
                load_w_pair(kp)
                load_x_pair(0, kp)
            accs_a = [None] * TBH
            mm_chunk(0, 0, accs_a)
            for kp in range(KPC, NKP):        # chunk 1 loads + signs + minis
                load_w_pair(kp)
                load_x_pair(0, kp)
            compute_scale()
            for kp in range(KPC):             # prefetch x th1 chunk0
                load_x_pair(1, kp)
            mm_chunk(0, 1, accs_a)
            for kp in range(KPC, NKP):        # prefetch x th1 chunk1
                load_x_pair(1, kp)
            accs_b = [None] * TBH
            mm_chunk(1, 0, accs_b)
            mm_chunk(1, 1, accs_b)

    nc.compile()
    return nc


_NC_CACHE = None


def _get_nc():
    global _NC_CACHE
    if _NC_CACHE is None:
        _NC_CACHE = build_nc()
    return _NC_CACHE


def make_in_maps(x, weight):
    """Host-side shard + layout prep: per-core transposed bf16 k-tiles."""
    bf16 = ml_dtypes.bfloat16
    x = np.asarray(x, dtype=np.float32).reshape(T_FULL, I_FULL)
    weight = np.asarray(weight, dtype=np.float32)
    xts = []
    for ti in range(T_GRID):
        sh = x[ti * T_SH:(ti + 1) * T_SH]                 # [2048, 4096]
        xts.append(sh.T.astype(bf16, order="C").reshape(NK, P, T_SH))
    wts = []
    for oj in range(O_GRID):
        sh = weight[oj * O_SH:(oj + 1) * O_SH]
        wts.append(sh.T.astype(bf16, order="C").reshape(NK, P, O_SH))
    in_maps = []
    for core in range(8):
        ti, oj = core // O_GRID, core % O_GRID
        in_maps.append({"x": xts[ti], "w": wts[oj]})
    return in_maps


def kernel(x, weight):
    in_maps = make_in_maps(x, weight)
    nc = _get_nc()
    res = run_bass_kernel_spmd(nc, in_maps, list(range(8)))
    out = np.empty((T_FULL, O_FULL), dtype=np.float32)
    for core in range(8):
        ti, oj = core // O_GRID, core % O_GRID
        out[ti * T_SH:(ti + 1) * T_SH, oj * O_SH:(oj + 1) * O_SH] = (
            res.results[core]["out"]
        )
    return out.reshape(B, S, O_FULL)


# revision 16
# speedup vs baseline: 1.1477x; 1.1461x over previous
"""BiRealLinear Trainium2 kernel (host-tiled operands + fp8 DoubleRow).

Computes out = binact(x) @ quant_weight(w).T for
  x [4, 2048, 4096] f32, w [4096, 4096] f32  ->  out [4, 2048, 4096] f32

Forward semantics (STE parts drop out in forward):
  binact(x)       = sign(x)                      in {-1, 0, +1}
  quant_weight(w) = mean(|w|, axis=1) * sign(w)  per-output-row scale

So out[t, o] = scale[o] * sum_i sign(x[t,i]) * sign(w[o,i]).

8 cores in a 4 (token) x 2 (out-feature) grid; each core does a
[2048 x 4096 x 2048] sign-matmul. Host supplies both operands already
transposed (contraction dim i outermost) as bf16 k-tiles [32, 128, T] --
bf16 transport is exact for sign() and loses <1e-4 on mean|w| -- so the
device does no transposes at all. On-device per core:
  - plain contiguous DMA loads of the k-tiles,
  - ACT sign -> fp8e4 (+/-1 exact) pair-tiles [128, 2, T],
  - |w| = w * sign(w) (exact in bf16) summed into f32 partials for the
    scale row (finished with two tiny PE reductions/broadcasts),
  - PE matmul in fp8 DoubleRow mode: each MM contracts 256 (two k-tiles),
    PSUM f32 accumulation is exact (integer sums <= 4096),
  - contraction split in 2 chunks of 2048 with an exact fp16 SBUF
    accumulator so matmuls start before all operands are loaded,
  - drain = (chunk0_acc + PSUM) * scale_bc -> f32 out tiles -> DMA.
"""

import sys

import numpy as np

try:
    import concourse.bacc as bacc  # noqa: F401
except ImportError:
    sys.path.insert(0, "/opt/trn_rl_repo")

import ml_dtypes

import concourse.bacc as bacc
import concourse.mybir as mybir
import concourse.tile as tile
from concourse.bass_utils import run_bass_kernel_spmd

dt = mybir.dt

# ---- problem geometry (hardcoded; full input is [8192, 4096] x [4096, 4096])
B, S, I_FULL, O_FULL = 4, 2048, 4096, 4096
T_FULL = B * S                      # 8192 tokens
T_GRID, O_GRID = 4, 2               # core grid: 4 token shards x 2 out shards
T_SH = T_FULL // T_GRID             # 2048 tokens per core
O_SH = O_FULL // O_GRID             # 2048 out features per core

P = 128                             # partitions
NK = I_FULL // P                    # 32 k-tiles
NKP = NK // 2                       # 16 k-pairs (DoubleRow contracts 2 tiles)
NCH = 2                             # contraction chunks (k-pairs 0-7 / 8-15)
KPC = NKP // NCH                    # 8 k-pairs per chunk
NTH = 2                             # token halves
TH = T_SH // NTH                    # 1024 tokens per half
NTB = T_SH // P                     # 16 t-blocks
TBH = NTB // NTH                    # 8 t-blocks per half
OP = 512                            # output panel (one PSUM bank)
NPAN = O_SH // OP                   # 4 panels


def build_nc():
    nc = bacc.Bacc("TRN2", target_bir_lowering=False, debug=False, num_devices=8)
    x = nc.dram_tensor("x", [NK, P, T_SH], dt.bfloat16, kind="ExternalInput")
    w = nc.dram_tensor("w", [NK, P, O_SH], dt.bfloat16, kind="ExternalInput")
    out = nc.dram_tensor("out", [T_SH, O_SH], dt.float32, kind="ExternalOutput")

    with tile.TileContext(nc) as tc:
        with (
            tc.tile_pool(name="single", bufs=1) as sb,
            tc.tile_pool(name="wstg", bufs=4) as wstg,
            tc.tile_pool(name="xstg", bufs=8) as xstg,
            tc.tile_pool(name="wabs", bufs=2) as wabp,
            tc.tile_pool(name="wp", bufs=NKP) as wpp,
            tc.tile_pool(name="xp", bufs=20) as xpp,
            tc.tile_pool(name="acc", bufs=TBH) as accp,
            tc.tile_pool(name="ostg", bufs=6) as ostgp,
            tc.tile_pool(name="ps", bufs=6, space="PSUM") as psp,
            tc.tile_pool(name="pss", bufs=2, space="PSUM") as pssp,
        ):
            ones_inv = sb.tile([P, 1], dt.bfloat16)     # 1/I column (k-reduce)
            nc.vector.memset(ones_inv[:], 1.0 / I_FULL)
            ones_row = sb.tile([1, P], dt.float32)      # 1.0 row (broadcast)
            nc.vector.memset(ones_row[:], 1.0)
            srow = sb.tile([1, O_SH], dt.float32)       # scale row
            scale_bc = sb.tile([P, O_SH], dt.float32)   # scale bcast to 128p
            # |w| column sums accumulate in ONE psum bank: the reduce matmul
            # for o-panel p writes M=1 rows at partition 32p (tile_position).
            pscale = pssp.tile([P, OP], dt.float32, tag="pss", name="pscale")

            WP = [None] * NKP                 # fp8 w pair-tiles, all resident
            XP = {}                           # (th, kp) -> fp8 x pair-tile

            def load_w_pair(kp):
                wp_t = wpp.tile([P, 2, O_SH], dt.float8e4, tag="wp",
                                name=f"wp_{kp}")
                for j in range(2):
                    k = 2 * kp + j
                    wtb = wstg.tile([P, O_SH], dt.bfloat16, tag="wstg",
                                    name=f"wtb_{k}")
                    # w on the SWDGE ring: the sync ring carries only x, so
                    # deep x staging cannot starve the w stream.
                    nc.gpsimd.dma_start(wtb[:], w[k])
                    nc.scalar.sign(wp_t[:, j, :], wtb[:])
                    wab = wabp.tile([P, O_SH], dt.bfloat16, tag="wabs",
                                    name=f"wab_{k}")
                    nc.vector.tensor_mul(wab[:], wtb[:], wp_t[:, j, :])
                    # scale partials: pscale[32p, o] += (1/I) * sum_i |w|
                    for p in range(NPAN):
                        nc.tensor.matmul(
                            pscale[32 * p:32 * p + 1, :],
                            lhsT=ones_inv[:],
                            rhs=wab[:, p * OP:(p + 1) * OP],
                            start=(k == 0), stop=(k == NK - 1),
                            tile_position=(0, 32 * p),
                            skip_group_check=True,
                        )
                WP[kp] = wp_t

            def load_x_pair(th, kp):
                xp_t = xpp.tile([P, 2, TH], dt.float8e4, tag="xp",
                                name=f"xp_{th}_{kp}")
                for j in range(2):
                    k = 2 * kp + j
                    xtb = xstg.tile([P, TH], dt.bfloat16, tag="xstg",
                                    name=f"xtb_{th}_{k}")
                    nc.sync.dma_start(xtb[:], x[k][:, th * TH:(th + 1) * TH])
                    nc.scalar.sign(xp_t[:, j, :], xtb[:])
                XP[(th, kp)] = xp_t

            def compute_scale():
                # pscale rows 0/32/64/96 hold the per-panel scale rows;
                # broadcast each to 128 partitions via a K=1 ones matmul.
                for p in range(NPAN):
                    nc.scalar.copy(srow[:, p * OP:(p + 1) * OP],
                                   pscale[32 * p:32 * p + 1, :])
                for p in range(NPAN):
                    psb = pssp.tile([P, OP], dt.float32, tag="pss",
                                    name=f"pssb_{p}")
                    nc.tensor.matmul(psb[:], lhsT=ones_row[:],
                                     rhs=srow[0:1, p * OP:(p + 1) * OP],
                                     start=True, stop=True)
                    nc.vector.tensor_copy(scale_bc[:, p * OP:(p + 1) * OP],
                                          psb[:])

            def mm_chunk(th, c, accs):
                first, last = (c == 0), (c == NCH - 1)
                for tbl in range(TBH):
                    tb = th * TBH + tbl
                    pst = [psp.tile([P, OP], dt.float32, tag="ps",
                                    name=f"ps_{tb}_{c}_{p}")
                           for p in range(NPAN)]
                    for kpl in range(KPC):
                        kp = c * KPC + kpl
                        lhsT = XP[(th, kp)][:, :, tbl * P:(tbl + 1) * P]
                        for p in range(NPAN):
                            nc.tensor.matmul(
                                pst[p][:], lhsT=lhsT,
                                rhs=WP[kp][:, :, p * OP:(p + 1) * OP],
                                start=(kpl == 0), stop=(kpl == KPC - 1),
                                perf_mode=mybir.MatmulPerfMode.DoubleRow,
                            )
                    if first:
                        acc = accp.tile([P, O_SH], dt.float16, tag="acc",
                                        name=f"acc_{tb}")
                        accs[tbl] = acc
                        for p in range(NPAN):
                            nc.vector.tensor_copy(acc[:, p * OP:(p + 1) * OP],
                                                  pst[p][:])
                    if last:
                        acc = accs[tbl]
                        for p in range(NPAN):
                            og = ostgp.tile([P, OP], dt.float32, tag="ostg",
                                            name=f"og_{tb}_{p}")
                            nc.vector.tensor_add(og[:], pst[p][:],
                                                 acc[:, p * OP:(p + 1) * OP])
                            nc.vector.tensor_mul(og[:], og[:],
                                                 scale_bc[:, p * OP:(p + 1) * OP])
                            nc.scalar.dma_start(
                                out[tb * P:(tb + 1) * P, p * OP:(p + 1) * OP],
                                og[:])

            # ---- program order: loads/signs feed the PE just ahead of use.
            # PE order is [minis c0][A-c0][minis c1][bcast][A-c1][B-c0][B-c1]
            # so the scale minis backfill the ACT-gated load window and the
            # chunk-1 minis/loads issue behind the A-c0 mains.
            for kp in range(KPC):             # chunk 0 loads + signs + minis
                load_w_pair(kp)
                load_x_pair(0, kp)
            accs_a = [None] * TBH
            mm_chunk(0, 0, accs_a)
            for kp in range(KPC, NKP):        # chunk 1 loads + signs + minis
                load_w_pair(kp)
                load_x_pair(0, kp)
            compute_scale()
            for kp in range(KPC):             # prefetch x th1 chunk0
                load_x_pair(1, kp)
            mm_chunk(0, 1, accs_a)
            for kp in range(KPC, NKP):        # prefetch x th1 chunk1
                load_x_pair(1, kp)
            accs_b = [None] * TBH
            mm_chunk(1, 0, accs_b)
            mm_chunk(1, 1, accs_b)

    nc.compile()
    return nc


_NC_CACHE = None


def _get_nc():
    global _NC_CACHE
    if _NC_CACHE is None:
        _NC_CACHE = build_nc()
    return _NC_CACHE


def make_in_maps(x, weight):
    """Host-side shard + layout prep: per-core transposed bf16 k-tiles."""
    bf16 = ml_dtypes.bfloat16
    x = np.asarray(x, dtype=np.float32).reshape(T_FULL, I_FULL)
    weight = np.asarray(weight, dtype=np.float32)
    xts = []
    for ti in range(T_GRID):
        sh = x[ti * T_SH:(ti + 1) * T_SH]                 # [2048, 4096]
        xts.append(sh.T.astype(bf16, order="C").reshape(NK, P, T_SH))
    wts = []
    for oj in range(O_GRID):
        sh = weight[oj * O_SH:(oj + 1) * O_SH]
        wts.append(sh.T.astype(bf16, order="C").reshape(NK, P, O_SH))
    in_maps = []
    for core in range(8):
        ti, oj = core // O_GRID, core % O_GRID
        in_maps.append({"x": xts[ti], "w": wts[oj]})
    return in_maps


def kernel(x, weight):
    in_maps = make_in_maps(x, weight)
    nc = _get_nc()
    res = run_bass_kernel_spmd(nc, in_maps, list(range(8)))
    out = np.empty((T_FULL, O_FULL), dtype=np.float32)
    for core in range(8):
        ti, oj = core // O_GRID, core % O_GRID
        out[ti * T_SH:(ti + 1) * T_SH, oj * O_SH:(oj + 1) * O_SH] = (
            res.results[core]["out"]
        )
    return out.reshape(B, S, O_FULL)


# revision 17
# speedup vs baseline: 1.1977x; 1.0436x over previous
"""BiRealLinear Trainium2 kernel (host-tiled operands + fp8 DoubleRow).

Computes out = binact(x) @ quant_weight(w).T for
  x [4, 2048, 4096] f32, w [4096, 4096] f32  ->  out [4, 2048, 4096] f32

Forward semantics (STE parts drop out in forward):
  binact(x)       = sign(x)                      in {-1, 0, +1}
  quant_weight(w) = mean(|w|, axis=1) * sign(w)  per-output-row scale

So out[t, o] = scale[o] * sum_i sign(x[t,i]) * sign(w[o,i]).

8 cores in a 4 (token) x 2 (out-feature) grid; each core does a
[2048 x 4096 x 2048] sign-matmul. Host supplies both operands already
transposed (contraction dim i outermost) as bf16 k-tiles [32, 128, T] --
bf16 transport is exact for sign() and loses <1e-4 on mean|w| -- so the
device does no transposes at all. On-device per core:
  - plain contiguous DMA loads of the k-tiles,
  - ACT sign -> fp8e4 (+/-1 exact) pair-tiles [128, 2, T],
  - |w| = w * sign(w) (exact in bf16) summed into f32 partials for the
    scale row (finished with two tiny PE reductions/broadcasts),
  - PE matmul in fp8 DoubleRow mode: each MM contracts 256 (two k-tiles),
    PSUM f32 accumulation is exact (integer sums <= 4096),
  - contraction split in 2 chunks of 2048 with an exact fp16 SBUF
    accumulator so matmuls start before all operands are loaded,
  - drain = (chunk0_acc + PSUM) * scale_bc -> f32 out tiles -> DMA.
"""

import sys

import numpy as np

try:
    import concourse.bacc as bacc  # noqa: F401
except ImportError:
    sys.path.insert(0, "/opt/trn_rl_repo")

import ml_dtypes

import concourse.bacc as bacc
import concourse.mybir as mybir
import concourse.tile as tile
from concourse.bass_utils import run_bass_kernel_spmd

dt = mybir.dt

# ---- problem geometry (hardcoded; full input is [8192, 4096] x [4096, 4096])
B, S, I_FULL, O_FULL = 4, 2048, 4096, 4096
T_FULL = B * S                      # 8192 tokens
T_GRID, O_GRID = 4, 2               # core grid: 4 token shards x 2 out shards
T_SH = T_FULL // T_GRID             # 2048 tokens per core
O_SH = O_FULL // O_GRID             # 2048 out features per core

P = 128                             # partitions
NK = I_FULL // P                    # 32 k-tiles
NKP = NK // 2                       # 16 k-pairs (DoubleRow contracts 2 tiles)
NCH = 2                             # contraction chunks (k-pairs 0-7 / 8-15)
KPC = NKP // NCH                    # 8 k-pairs per chunk
NTH = 2                             # token halves
TH = T_SH // NTH                    # 1024 tokens per half
NTB = T_SH // P                     # 16 t-blocks
TBH = NTB // NTH                    # 8 t-blocks per half
OP = 512                            # output panel (one PSUM bank)
NPAN = O_SH // OP                   # 4 panels


def build_nc():
    nc = bacc.Bacc("TRN2", target_bir_lowering=False, debug=False, num_devices=8)
    x = nc.dram_tensor("x", [NK, P, T_SH], dt.bfloat16, kind="ExternalInput")
    w = nc.dram_tensor("w", [NK, P, O_SH], dt.bfloat16, kind="ExternalInput")
    out = nc.dram_tensor("out", [T_SH, O_SH], dt.float32, kind="ExternalOutput")

    with tile.TileContext(nc) as tc:
        with (
            tc.tile_pool(name="single", bufs=1) as sb,
            tc.tile_pool(name="wstg", bufs=4) as wstg,
            tc.tile_pool(name="xstg", bufs=8) as xstg,
            tc.tile_pool(name="wabs", bufs=2) as wabp,
            tc.tile_pool(name="wp", bufs=NKP) as wpp,
            tc.tile_pool(name="xp", bufs=20) as xpp,
            tc.tile_pool(name="acc", bufs=TBH + 2) as accp,
            tc.tile_pool(name="ostg", bufs=8) as ostgp,
            tc.tile_pool(name="ps", bufs=6, space="PSUM") as psp,
            tc.tile_pool(name="pss", bufs=2, space="PSUM") as pssp,
        ):
            ones_inv = sb.tile([P, 1], dt.bfloat16)     # 1/I column (k-reduce)
            nc.vector.memset(ones_inv[:], 1.0 / I_FULL)
            ones_row = sb.tile([1, P], dt.float32)      # 1.0 row (broadcast)
            nc.vector.memset(ones_row[:], 1.0)
            srow = sb.tile([1, O_SH], dt.float32)       # scale row
            scale_bc = sb.tile([P, O_SH], dt.float32)   # scale bcast to 128p
            # |w| column sums accumulate in ONE psum bank: the reduce matmul
            # for o-panel p writes M=1 rows at partition 32p (tile_position).
            pscale = pssp.tile([P, OP], dt.float32, tag="pss", name="pscale")

            WP = [None] * NKP                 # fp8 w pair-tiles, all resident
            XP = {}                           # (th, kp) -> fp8 x pair-tile

            def load_w_pair(kp):
                wp_t = wpp.tile([P, 2, O_SH], dt.float8e4, tag="wp",
                                name=f"wp_{kp}")
                for j in range(2):
                    k = 2 * kp + j
                    wtb = wstg.tile([P, O_SH], dt.bfloat16, tag="wstg",
                                    name=f"wtb_{k}")
                    # w on the SWDGE ring: the sync ring carries only x, so
                    # deep x staging cannot starve the w stream.
                    nc.gpsimd.dma_start(wtb[:], w[k])
                    nc.scalar.sign(wp_t[:, j, :], wtb[:])
                    wab = wabp.tile([P, O_SH], dt.bfloat16, tag="wabs",
                                    name=f"wab_{k}")
                    nc.vector.tensor_mul(wab[:], wtb[:], wp_t[:, j, :])
                    # scale partials: pscale[32p, o] += (1/I) * sum_i |w|
                    for p in range(NPAN):
                        nc.tensor.matmul(
                            pscale[32 * p:32 * p + 1, :],
                            lhsT=ones_inv[:],
                            rhs=wab[:, p * OP:(p + 1) * OP],
                            start=(k == 0), stop=(k == NK - 1),
                            tile_position=(0, 32 * p),
                            skip_group_check=True,
                        )
                WP[kp] = wp_t

            def load_x_pair(th, kp):
                xp_t = xpp.tile([P, 2, TH], dt.float8e4, tag="xp",
                                name=f"xp_{th}_{kp}")
                for j in range(2):
                    k = 2 * kp + j
                    xtb = xstg.tile([P, TH], dt.bfloat16, tag="xstg",
                                    name=f"xtb_{th}_{k}")
                    nc.sync.dma_start(xtb[:], x[k][:, th * TH:(th + 1) * TH])
                    nc.scalar.sign(xp_t[:, j, :], xtb[:])
                XP[(th, kp)] = xp_t

            def compute_scale():
                # pscale rows 0/32/64/96 hold the per-panel scale rows;
                # broadcast each to 128 partitions via a K=1 ones matmul.
                for p in range(NPAN):
                    nc.scalar.copy(srow[:, p * OP:(p + 1) * OP],
                                   pscale[32 * p:32 * p + 1, :])
                for p in range(NPAN):
                    psb = pssp.tile([P, OP], dt.float32, tag="pss",
                                    name=f"pssb_{p}")
                    nc.tensor.matmul(psb[:], lhsT=ones_row[:],
                                     rhs=srow[0:1, p * OP:(p + 1) * OP],
                                     start=True, stop=True)
                    nc.vector.tensor_copy(scale_bc[:, p * OP:(p + 1) * OP],
                                          psb[:])

            def mm_chunk(th, c, accs):
                first, last = (c == 0), (c == NCH - 1)
                for tbl in range(TBH):
                    tb = th * TBH + tbl
                    pst = [psp.tile([P, OP], dt.float32, tag="ps",
                                    name=f"ps_{tb}_{c}_{p}")
                           for p in range(NPAN)]
                    for kpl in range(KPC):
                        kp = c * KPC + kpl
                        lhsT = XP[(th, kp)][:, :, tbl * P:(tbl + 1) * P]
                        for p in range(NPAN):
                            nc.tensor.matmul(
                                pst[p][:], lhsT=lhsT,
                                rhs=WP[kp][:, :, p * OP:(p + 1) * OP],
                                start=(kpl == 0), stop=(kpl == KPC - 1),
                                perf_mode=mybir.MatmulPerfMode.DoubleRow,
                            )
                    if first:
                        acc = accp.tile([P, O_SH], dt.float16, tag="acc",
                                        name=f"acc_{tb}")
                        accs[tbl] = acc
                        for p in range(NPAN):
                            nc.vector.tensor_copy(acc[:, p * OP:(p + 1) * OP],
                                                  pst[p][:])
                    if last:
                        acc = accs[tbl]
                        for p in range(NPAN):
                            og = ostgp.tile([P, OP], dt.float32, tag="ostg",
                                            name=f"og_{tb}_{p}")
                            nc.vector.tensor_add(og[:], pst[p][:],
                                                 acc[:, p * OP:(p + 1) * OP])
                            nc.vector.tensor_mul(og[:], og[:],
                                                 scale_bc[:, p * OP:(p + 1) * OP])
                            nc.scalar.dma_start(
                                out[tb * P:(tb + 1) * P, p * OP:(p + 1) * OP],
                                og[:])

            # ---- program order: loads/signs feed the PE just ahead of use.
            # PE order is [minis c0][A-c0][minis c1][bcast][A-c1][B-c0][B-c1]
            # so the scale minis backfill the ACT-gated load window and the
            # chunk-1 minis/loads issue behind the A-c0 mains.
            for kp in range(KPC):             # chunk 0 loads + signs + minis
                load_w_pair(kp)
                load_x_pair(0, kp)
            accs_a = [None] * TBH
            mm_chunk(0, 0, accs_a)
            for kp in range(KPC, NKP):        # chunk 1 loads + signs + minis
                load_w_pair(kp)
                load_x_pair(0, kp)
            compute_scale()
            for kp in range(KPC):             # prefetch x th1 chunk0
                load_x_pair(1, kp)
            mm_chunk(0, 1, accs_a)
            for kp in range(KPC, NKP):        # prefetch x th1 chunk1
                load_x_pair(1, kp)
            accs_b = [None] * TBH
            mm_chunk(1, 0, accs_b)
            mm_chunk(1, 1, accs_b)

    nc.compile()
    return nc


_NC_CACHE = None


def _get_nc():
    global _NC_CACHE
    if _NC_CACHE is None:
        _NC_CACHE = build_nc()
    return _NC_CACHE


def make_in_maps(x, weight):
    """Host-side shard + layout prep: per-core transposed bf16 k-tiles."""
    bf16 = ml_dtypes.bfloat16
    x = np.asarray(x, dtype=np.float32).reshape(T_FULL, I_FULL)
    weight = np.asarray(weight, dtype=np.float32)
    xts = []
    for ti in range(T_GRID):
        sh = x[ti * T_SH:(ti + 1) * T_SH]                 # [2048, 4096]
        xts.append(sh.T.astype(bf16, order="C").reshape(NK, P, T_SH))
    wts = []
    for oj in range(O_GRID):
        sh = weight[oj * O_SH:(oj + 1) * O_SH]
        wts.append(sh.T.astype(bf16, order="C").reshape(NK, P, O_SH))
    in_maps = []
    for core in range(8):
        ti, oj = core // O_GRID, core % O_GRID
        in_maps.append({"x": xts[ti], "w": wts[oj]})
    return in_maps


def kernel(x, weight):
    in_maps = make_in_maps(x, weight)
    nc = _get_nc()
    res = run_bass_kernel_spmd(nc, in_maps, list(range(8)))
    out = np.empty((T_FULL, O_FULL), dtype=np.float32)
    for core in range(8):
        ti, oj = core // O_GRID, core % O_GRID
        out[ti * T_SH:(ti + 1) * T_SH, oj * O_SH:(oj + 1) * O_SH] = (
            res.results[core]["out"]
        )
    return out.reshape(B, S, O_FULL)
